# revision 1
# baseline (speedup 1.0000x reference)
"""v2: all-feature-major Bengio03ResNetBiLm kernel.

Key differences from v1:
- every matmul is weight-stationary with N=512 moving tokens (LDWEIGHTS fully
  hidden behind the 213ns fp32r stream), psum outputs feature-major;
- layernorm is computed via partition-reduction matmuls (ones stationary,
  M=1 rows placed at psum partition 32g per 512-token group) and applied by
  folding: relu(n@w1'+b1') == relu((z + (-m) x cs1) * rstd_bcast + b1') where
  z = h@w1', cs1 = colsum(w1') — the mean correction is a rank-1 matmul into
  the same psum and rstd arrives via a broadcast matmul;
- ffn2 bias + residual fuse into one scalar_tensor_tensor op;
- only remaining transposes: the token-major store path + x0 ingestion.
"""

import contextlib

import numpy as np

import concourse.bacc as bacc
import concourse.tile as tile
from concourse import mybir
from concourse.masks import make_identity

F32 = mybir.dt.float32
F32R = mybir.dt.float32r
BF16 = mybir.dt.bfloat16
AF = mybir.ActivationFunctionType
ALU = mybir.AluOpType

W = 3
H = 256
HC = 2
EPS = 1e-6


def prep_weights(inputs, L):
    f32 = np.float32
    LB = 2 * L
    wpT = np.zeros((L, 2, 4, HC, 128, HC, 128), f32)  # [l, br, j, c, p, m, n]
    ctxb_col = np.zeros((LB, 128, HC), f32)
    w1pT = np.zeros((L, 2, HC, 128, HC, 128), f32)    # [l, br, c, p, m, n]
    b1p = np.zeros((LB, 128, HC), f32)
    cs1_s = np.zeros((128, LB, HC, 128), f32)          # rows 32g = colsum(w1')
    w2T = np.zeros((L, 2, HC, 128, HC, 128), f32)
    b2col = np.zeros((LB, 128, HC), f32)
    padT = np.zeros((L, HC, 128, 2 * W), f32)

    for l in range(L):
        for br, (Wc, bc, g, beta, w1, b1, w2_, b2) in enumerate(
            (
                (inputs["fwd_W"][l], inputs["fwd_b"][l], inputs["ln_f_g"][l],
                 inputs["ln_f_b"][l], inputs["ffn_f_w1"][l], inputs["ffn_f_b1"][l],
                 inputs["ffn_f_w2"][l], inputs["ffn_f_b2"][l]),
                (inputs["bwd_W"][l], inputs["bwd_b"][l], inputs["ln_b_g"][l],
                 inputs["ln_b_b"][l], inputs["ffn_b_w1"][l], inputs["ffn_b_b1"][l],
                 inputs["ffn_b_w2"][l], inputs["ffn_b_b2"][l]),
            )
        ):
            lb = l * 2 + br
            wpT[l, br] = np.asarray(Wc, f32).reshape(4, HC, 128, HC, 128)
            ctxb_col[lb] = np.asarray(bc, f32).reshape(HC, 128).T
            w1f = np.asarray(g, f32)[:, None] * np.asarray(w1, f32)
            b1f = np.asarray(b1, f32) + np.asarray(beta, f32) @ np.asarray(w1, f32)
            w1pT[l, br] = w1f.reshape(HC, 128, HC, 128)
            b1p[lb] = b1f.reshape(HC, 128).T
            cs1 = w1f.sum(0).reshape(HC, 128)  # colsum
            for gg in range(4):
                cs1_s[32 * gg, lb] = cs1
            w2T[l, br] = np.asarray(w2_, f32).reshape(HC, 128, HC, 128)
            b2col[lb] = np.asarray(b2, f32).reshape(HC, 128).T
        fp = np.asarray(inputs["fwd_pad"][l], f32)
        bp = np.asarray(inputs["bwd_pad"][l], f32)
        padT[l] = np.concatenate([fp, bp], 0).T.reshape(HC, 128, 2 * W)

    ones4 = np.zeros((128, 128), f32)
    for gg in range(4):
        ones4[32 * gg] = 1.0
    ohcols = np.zeros((4, 128, 128), f32)
    for gg in range(4):
        ohcols[gg, :, 32 * gg] = 1.0
    return dict(wpT=wpT, ctxb_col=ctxb_col, w1pT=w1pT, b1p=b1p, cs1_s=cs1_s,
                w2T=w2T, b2col=b2col, padT=padT, ones4=ones4, ohcols=ohcols)


def build_nc(B_local, S_, L, arsqrt=True, mm_dt=F32R):
    NG = S_ // 512
    SP = S_ + 2 * W
    LB = 2 * L

    nc = bacc.Bacc()
    dr = {}
    dr["x0"] = nc.dram_tensor("x0", [B_local, S_, H], F32, kind="ExternalInput")
    dr["wpT"] = nc.dram_tensor("wpT", [L, 2, 4, HC, 128, HC, 128], F32,
                               kind="ExternalInput")
    dr["ctxb_col"] = nc.dram_tensor("ctxb_col", [LB, 128, HC], F32,
                                    kind="ExternalInput")
    dr["w1pT"] = nc.dram_tensor("w1pT", [L, 2, HC, 128, HC, 128], F32,
                                kind="ExternalInput")
    dr["b1p"] = nc.dram_tensor("b1p", [LB, 128, HC], F32, kind="ExternalInput")
    dr["cs1_s"] = nc.dram_tensor("cs1_s", [128, LB, HC, 128], F32,
                                 kind="ExternalInput")
    dr["w2T"] = nc.dram_tensor("w2T", [L, 2, HC, 128, HC, 128], F32,
                               kind="ExternalInput")
    dr["b2col"] = nc.dram_tensor("b2col", [LB, 128, HC], F32, kind="ExternalInput")
    dr["padT"] = nc.dram_tensor("padT", [L, HC, 128, 2 * W], F32,
                                kind="ExternalInput")
    dr["ones4"] = nc.dram_tensor("ones4", [128, 128], F32, kind="ExternalInput")
    dr["ohcols"] = nc.dram_tensor("ohcols", [4, 128, 128], F32, kind="ExternalInput")
    dr["out"] = nc.dram_tensor("out", [L, B_local, S_, 2 * H], F32,
                               kind="ExternalOutput")

    with tile.TileContext(nc) as tc:
        _body(nc, tc, B_local, S_, L, NG, SP, LB, dr, arsqrt, mm_dt)
    nc.compile()
    return nc


def _body(nc, tc, B_local, S_, L, NG, SP, LB, dr, arsqrt=True, MMDT=F32R):
    ctx = contextlib.ExitStack()
    with ctx:
        consts = ctx.enter_context(tc.tile_pool(name="consts", bufs=1))
        wstream = ctx.enter_context(tc.tile_pool(name="wstream", bufs=2))
        xbufs = ctx.enter_context(tc.tile_pool(name="xbufs", bufs=1))
        x0tm_p = ctx.enter_context(tc.tile_pool(name="x0tm", bufs=2))
        h_p = ctx.enter_context(tc.tile_pool(name="h", bufs=2))
        sq_p = ctx.enter_context(tc.tile_pool(name="sq", bufs=2))
        rows_p = ctx.enter_context(tc.tile_pool(name="rows", bufs=2))
        f1_p = ctx.enter_context(tc.tile_pool(name="f1", bufs=2))
        tmp_p = ctx.enter_context(tc.tile_pool(name="tmp", bufs=2))
        xn_p = ctx.enter_context(tc.tile_pool(name="xn", bufs=2))
        tm_p = ctx.enter_context(tc.tile_pool(name="tm", bufs=2))
        pm = ctx.enter_context(tc.tile_pool(name="pm", bufs=5, space="PSUM"))
        ps_st = ctx.enter_context(tc.tile_pool(name="ps_st", bufs=1, space="PSUM"))
        ps_misc = ctx.enter_context(tc.tile_pool(name="ps_misc", bufs=1, space="PSUM"))

        # ---- constants ----
        ident = consts.tile([128, 128], F32)
        make_identity(nc, ident[:])
        ident_r = consts.tile([128, 128], MMDT)
        nc.vector.tensor_copy(out=ident_r[:], in_=ident[:])
        eps_t = consts.tile([128, 1], F32)
        nc.vector.memset(eps_t[:], EPS)
        ones4 = consts.tile([128, 128], MMDT)
        nc.gpsimd.dma_start(ones4[:], dr["ones4"].ap())
        ohcols = consts.tile([128, 4, 128], MMDT)
        nc.gpsimd.dma_start(ohcols[:], dr["ohcols"].ap().rearrange("g p m -> p g m"))
        cs1_s = consts.tile([128, LB, HC, 128], MMDT)
        nc.gpsimd.dma_start(cs1_s[:], dr["cs1_s"].ap())
        ctxb_col = consts.tile([128, LB, HC], F32)
        nc.sync.dma_start(ctxb_col[:], dr["ctxb_col"].ap().rearrange("a p m -> p a m"))
        b1p = consts.tile([128, LB, HC], F32)
        nc.sync.dma_start(b1p[:], dr["b1p"].ap().rearrange("a p m -> p a m"))
        b2col = consts.tile([128, LB, HC], F32)
        nc.sync.dma_start(b2col[:], dr["b2col"].ap().rearrange("a p m -> p a m"))
        padT_s = consts.tile([128, L, HC, 2 * W], MMDT)
        nc.gpsimd.dma_start(padT_s[:], dr["padT"].ap().rearrange("l c p w -> p l c w"))
        w1pT_s = consts.tile([128, L, 2, HC, HC, 128], MMDT)
        nc.gpsimd.dma_start(
            w1pT_s[:], dr["w1pT"].ap().rearrange("l b c p m n -> p l b c m n"))
        w2T_s = consts.tile([128, L, 2, HC, HC, 128], MMDT)
        nc.gpsimd.dma_start(
            w2T_s[:], dr["w2T"].ap().rearrange("l b c p m n -> p l b c m n"))

        def cp_act(dst, srcap):
            nc.scalar.copy(out=dst, in_=srcap)

        def cp_dve(dst, srcap):
            nc.vector.tensor_copy(out=dst, in_=srcap)

        copy_fns = [cp_act, cp_dve]

        # ---- per-batch buffer state ----
        bufs = {}   # b -> dict(x0, xA, xB)
        wp_tiles = {}

        def in_buf(b, l, br):
            d = bufs[b]
            return d["x0"] if l == 0 else (d["xA"][br] if l % 2 == 1 else d["xB"][br])

        def out_buf(b, l, br):
            d = bufs[b]
            return d["xA"][br] if l % 2 == 0 else d["xB"][br]

        # ---- unit emission helpers; unit = (b, l, br) ----
        state = {}  # unit -> dict(h_sb, st_sum, st_sq, negm, rstd)

        def prologue(u):
            b, l, br = u
            if l == 0 and br == 0:
                d = {}
                d["x0"] = xbufs.tile([128, HC, SP], MMDT, tag="xB0",
                                     name=f"x0_fm_{b}")
                d["xA"] = [xbufs.tile([128, HC, SP], MMDT, tag=f"xA{i}",
                                      name=f"xA{i}_{b}") for i in range(2)]
                d["xB"] = [xbufs.tile([128, HC, SP], MMDT, tag=f"xB{i}",
                                      name=f"xB{i}_{b}") for i in range(2)]
                bufs[b] = d
                x0_fm = d["x0"]
                for g2 in range(S_ // 128):
                    x0tm = x0tm_p.tile([128, H], F32, tag="x0tm", name="x0tm")
                    nc.sync.dma_start(
                        x0tm[:], dr["x0"].ap()[b, g2 * 128 : (g2 + 1) * 128, :])
                    pst = ps_misc.tile([128, 2, 128], F32, tag="misc", name="pst0")
                    for c in range(HC):
                        nc.tensor.transpose(
                            pst[:, c, :], x0tm[:, c * 128 : (c + 1) * 128], ident[:])
                        col = W + g2 * 128
                        copy_fns[c](x0_fm[:, c, col : col + 128], pst[:, c, :])
            if br == 0:
                wpT = wstream.tile([128, 2, 4, HC, HC, 128], MMDT, tag="wpT",
                                   name=f"wpT_{b}_{l}")
                nc.gpsimd.dma_start(
                    wpT[:],
                    dr["wpT"].ap()[l].rearrange("b j c p m n -> p b j c m n"))
                wp_tiles[(b, l)] = wpT
            if not (l == 0 and br == 1):
                buf = in_buf(b, l, br)
                for c in range(HC):
                    nc.gpsimd.tensor_copy(buf[:, c, 0:W], padT_s[:, l, c, 0:W])
                    nc.gpsimd.tensor_copy(
                        buf[:, c, S_ + W : S_ + 2 * W], padT_s[:, l, c, W : 2 * W])

        def emit_A(u, g):
            b, l, br = u
            lb = l * 2 + br
            xin = in_buf(b, l, br)
            wpT = wp_tiles[(b, l)]
            off = 0 if br == 0 else W
            st = state.setdefault(u, {})
            if g == 0:
                st["h_sb"] = h_p.tile([128, HC, S_], MMDT, tag="h",
                                      name=f"h_{b}_{l}_{br}")
                st["st_sum"] = ps_st.tile([128, 512], F32, tag="st_sum",
                                          name=f"stsum_{b}_{l}_{br}")
                st["st_sq"] = ps_st.tile([128, 512], F32, tag="st_sq",
                                         name=f"stsq_{b}_{l}_{br}")
            h_sb = st["h_sb"]
            t0 = g * 512
            for m in range(HC):
                psc = pm.tile([128, 512], F32, tag="pm", name="psc")
                for j in range(W + 1):
                    for c in range(HC):
                        nc.tensor.matmul(
                            psc[:], wpT[:, br, j, c, m, :],
                            xin[:, c, t0 + off + j : t0 + off + j + 512],
                            start=(j == 0 and c == 0),
                            stop=(j == W and c == HC - 1))
                nc.scalar.activation(
                    h_sb[:, m, t0 : t0 + 512], psc[:], AF.Relu,
                    bias=ctxb_col[:, lb, m : m + 1])
            sqs = []
            for m in range(HC):
                sq = sq_p.tile([128, 512], MMDT, tag="sq", name="sq")
                nc.scalar.activation(sq[:], h_sb[:, m, t0 : t0 + 512], AF.Square)
                sqs.append(sq)
            for m in range(HC):
                nc.tensor.matmul(
                    st["st_sum"][:], ohcols[:, g % 4, :], h_sb[:, m, t0 : t0 + 512],
                    start=(g == 0 and m == 0), stop=(g == NG - 1 and m == HC - 1),
                    skip_group_check=True)
                nc.tensor.matmul(
                    st["st_sq"][:], ohcols[:, g % 4, :], sqs[m][:],
                    start=(g == 0 and m == 0), stop=(g == NG - 1 and m == HC - 1),
                    skip_group_check=True)

        def emit_R(u):
            st = state[u]
            negm = rows_p.tile([128, 512], MMDT, tag="negm", name="negm")
            nc.vector.tensor_scalar_mul(negm[:], st["st_sum"][:], -1.0 / H)
            m2 = rows_p.tile([128, 512], F32, tag="rtmp", name="m2")
            nc.vector.tensor_mul(m2[:], negm[:], negm[:])
            v = rows_p.tile([128, 512], F32, tag="rtmp", name="v")
            nc.vector.scalar_tensor_tensor(
                out=v[:], in0=st["st_sq"][:], scalar=1.0 / H, in1=m2[:],
                op0=ALU.mult, op1=ALU.subtract)
            rstd = rows_p.tile([128, 512], MMDT, tag="rstd", name="rstd")
            if arsqrt:
                nc.scalar.activation(
                    rstd[:], v[:], AF.Abs_reciprocal_sqrt, bias=eps_t[:])
            else:
                sig = rows_p.tile([128, 512], F32, tag="rtmp", name="sig")
                nc.scalar.activation(sig[:], v[:], AF.Sqrt, bias=eps_t[:])
                with nc.allow_low_precision(reason="f32r rstd for matmul rhs"):
                    nc.vector.reciprocal(rstd[:], sig[:])
            st["negm"] = negm
            st["rstd"] = rstd

        def emit_B1(u, g):
            b, l, br = u
            lb = l * 2 + br
            st = state[u]
            h_sb, negm, rstd = st["h_sb"], st["negm"], st["rstd"]
            t0 = g * 512
            gp = 32 * (g % 4)
            psz = [None, None]
            for m in range(HC):
                psz[m] = pm.tile([128, 512], F32, tag="pm", name="psz")
                for c in range(HC):
                    nc.tensor.matmul(
                        psz[m][:], w1pT_s[:, l, br, c, m, :],
                        h_sb[:, c, t0 : t0 + 512], start=(c == 0), stop=False)
                nc.tensor.matmul(
                    psz[m][:], cs1_s[gp : gp + 1, lb, m, :], negm[gp : gp + 1, :],
                    start=False, stop=True, tile_position=(gp, 0))
            rb = pm.tile([128, 512], F32, tag="pm", name="rb")
            nc.tensor.matmul(
                rb[:], ones4[gp : gp + 1, :], rstd[gp : gp + 1, :],
                start=True, stop=True, tile_position=(gp, 0))
            rb_sb = tmp_p.tile([128, 512], F32, tag="rb_sb", name="rb_sb")
            nc.scalar.copy(out=rb_sb[:], in_=rb[:])
            f1_sb = f1_p.tile([128, HC, 512], MMDT, tag="f1", name="f1_sb")
            for m in range(HC):
                t_sb = tmp_p.tile([128, 512], F32, tag="t_sb", name="t_sb")
                nc.vector.tensor_mul(t_sb[:], psz[m][:], rb_sb[:])
                nc.scalar.activation(
                    f1_sb[:, m, :], t_sb[:], AF.Relu, bias=b1p[:, lb, m : m + 1])
            st[("f1", g)] = f1_sb

        def emit_B2(u, g):
            b, l, br = u
            lb = l * 2 + br
            st = state[u]
            h_sb = st["h_sb"]
            f1_sb = st.pop(("f1", g))
            xout = out_buf(b, l, br) if l < L - 1 else None
            t0 = g * 512
            xn_st = xn_p.tile([128, HC, 512], MMDT, tag="xn", name="xn_st")
            for m in range(HC):
                pso = pm.tile([128, 512], F32, tag="pm", name="pso")
                for c in range(HC):
                    nc.tensor.matmul(
                        pso[:], w2T_s[:, l, br, c, m, :], f1_sb[:, c, :],
                        start=(c == 0), stop=(c == HC - 1))
                nc.vector.scalar_tensor_tensor(
                    out=xn_st[:, m, :], in0=pso[:],
                    scalar=b2col[:, lb, m : m + 1],
                    in1=h_sb[:, m, t0 : t0 + 512], op0=ALU.add, op1=ALU.add)
                if xout is not None:
                    nc.gpsimd.tensor_copy(
                        xout[:, m, W + t0 : W + t0 + 512], xn_st[:, m, :])
            for s in range(4):
                pst = ps_misc.tile([128, 2, 128], F32, tag="misc", name="pst")
                for m in range(HC):
                    nc.tensor.transpose(
                        pst[:, m, :].bitcast(MMDT),
                        xn_st[:, m, s * 128 : (s + 1) * 128],
                        ident_r[:])
                tmt = tm_p.tile([128, 2, 128], F32, tag="tmt", name="tmt")
                copy_fns[s % 2](tmt[:], pst[:])
                nc.sync.dma_start(
                    dr["out"].ap()[l, b, t0 + s * 128 : t0 + (s + 1) * 128,
                                   br * H : (br + 1) * H],
                    tmt[:])

        # ---- software-pipelined unit stream ----
        # Per 512-token group: A(u_i, g) | B1(u_{i-1}, g) | B2(u_{i-1}, g-1):
        # PE always has the next unit's context matmuls plus the previous
        # group's ffn2 available while B1's DVE/ACT chain produces f1.
        units = [(b, l, br) for b in range(B_local) for l in range(L)
                 for br in range(2)]
        prev = None
        for u in units:
            prologue(u)
            for g in range(NG):
                emit_A(u, g)
                if prev is not None:
                    emit_B1(prev, g)
                    if g > 0:
                        emit_B2(prev, g - 1)
            emit_R(u)
            if prev is not None:
                emit_B2(prev, NG - 1)
                state.pop(prev)
            prev = u
        for g in range(NG):
            emit_B1(prev, g)
            if g > 0:
                emit_B2(prev, g - 1)
        emit_B2(prev, NG - 1)


def ref_numpy(x0, inputs, L):
    B, S_, _ = x0.shape
    x_f = x_b = np.asarray(x0, np.float64)

    def branch(xpad, Wp, bp, g, beta, w1, b1, w2, b2, offs):
        ctxm = np.concatenate([xpad[:, k : k + S_] for k in offs], -1)
        h = np.maximum(ctxm @ Wp + bp, 0)
        m = h.mean(-1, keepdims=True)
        v = h.var(-1, keepdims=True)
        y = g * (h - m) / np.sqrt(v + EPS) + beta
        ffn = np.maximum(y @ w1 + b1, 0) @ w2 + b2
        return h + ffn

    outs = []
    I = {k: np.asarray(v, np.float64) for k, v in inputs.items() if k != "mask"}
    for l in range(L):
        fp = np.broadcast_to(I["fwd_pad"][l], (B, W, H))
        bp = np.broadcast_to(I["bwd_pad"][l], (B, W, H))
        pad_f = np.concatenate([fp, x_f, bp], 1)
        pad_b = np.concatenate([fp, x_b, bp], 1)
        x_f = branch(pad_f, I["fwd_W"][l], I["fwd_b"][l], I["ln_f_g"][l],
                     I["ln_f_b"][l], I["ffn_f_w1"][l], I["ffn_f_b1"][l],
                     I["ffn_f_w2"][l], I["ffn_f_b2"][l], range(W + 1))
        x_b = branch(pad_b, I["bwd_W"][l], I["bwd_b"][l], I["ln_b_g"][l],
                     I["ln_b_b"][l], I["ffn_b_w1"][l], I["ffn_b_b1"][l],
                     I["ffn_b_w2"][l], I["ffn_b_b2"][l], range(W, 2 * W + 1))
        outs.append(np.concatenate([x_f, x_b], -1))
    return np.stack(outs, 0)


# ---- SPMD wrapper ----
import numpy as np
from concourse.bass_utils import run_bass_kernel_spmd

B, S, L_ = 32, 2048, 3
N_CORES = 8
B_LOCAL = B // N_CORES
_NC_CACHE = {}


def _get_nc(arsqrt=True, mm_dt=F32R):
    key = (B_LOCAL, S, arsqrt, str(mm_dt))
    if key not in _NC_CACHE:
        _NC_CACHE[key] = build_nc(B_LOCAL, S, L_, arsqrt=arsqrt, mm_dt=mm_dt)
    return _NC_CACHE[key]


def run(inputs, arsqrt=True, mm_dt=F32R, **spmd_kwargs):
    prep = prep_weights(inputs, L_)
    x = np.ascontiguousarray(np.asarray(inputs["inputs"], np.float32))
    nc = _get_nc(arsqrt, mm_dt)
    in_maps = []
    for core in range(N_CORES):
        m = {"x0": x[core * B_LOCAL : (core + 1) * B_LOCAL]}
        m.update(prep)
        in_maps.append(m)
    res = run_bass_kernel_spmd(nc, in_maps, list(range(N_CORES)), **spmd_kwargs)
    out = np.concatenate([res.results[i]["out"] for i in range(N_CORES)], axis=1)
    return out, res


def kernel(**inputs):
    out, _ = run(inputs)
    return out



# revision 5
# speedup vs baseline: 1.3729x; 1.3729x over previous
"""v3: bf16 all-feature-major Bengio03ResNetBiLm kernel.

Changes vs v2 (f32r baseline, 1068us):
- bf16 activations/weights end-to-end (rel-err budget ~5e-3 << 2e-2);
  output DRAM tensor is bf16 (half the store traffic), x0 arrives
  pre-transposed to feature-major bf16 from the host (no ingest work);
- PE never stalls: LN-stats matmuls for group g are emitted one group
  late (their ACT/DVE producers have a full group of slack), and the
  per-unit row math (mean/rstd) hides behind the next unit's 3.4us
  context-projection stream;
- proj weights are loaded once per layer (not per (batch, layer));
- ffn2 output+residual is written straight into the next layer's input
  buffer; the store path transposes that buffer in bf16 (1.0 cyc/row)
  into one psum bank per group, evacuated by the Pool engine and
  shipped with a single DMA per 512-token group.
"""

import contextlib

import numpy as np
import ml_dtypes

import concourse.bacc as bacc
import concourse.tile as tile
from concourse import mybir
from concourse.masks import make_identity

F32 = mybir.dt.float32
BF16 = mybir.dt.bfloat16
AF = mybir.ActivationFunctionType
ALU = mybir.AluOpType

W = 3
H = 256
HC = 2
EPS = 1e-6
NPBF = ml_dtypes.bfloat16


def prep_weights(inputs, L):
    f32 = np.float32
    LB = 2 * L
    wpT = np.zeros((L, 2, 4, HC, 128, HC, 128), f32)  # [l, br, j, c, p, m, n]
    ctxb_col = np.zeros((LB, 128, HC), f32)
    w1pT = np.zeros((L, 2, HC, 128, HC, 128), f32)    # [l, br, c, p, m, n]
    b1p = np.zeros((LB, 128, HC), f32)
    cs1_s = np.zeros((128, LB, HC, 128), f32)          # rows 32g = colsum(w1')
    w2T = np.zeros((L, 2, HC, 128, HC, 128), f32)
    b2col = np.zeros((LB, 128, HC), f32)
    padT = np.zeros((L, HC, 128, 2 * W), f32)

    for l in range(L):
        for br, (Wc, bc, g, beta, w1, b1, w2_, b2) in enumerate(
            (
                (inputs["fwd_W"][l], inputs["fwd_b"][l], inputs["ln_f_g"][l],
                 inputs["ln_f_b"][l], inputs["ffn_f_w1"][l], inputs["ffn_f_b1"][l],
                 inputs["ffn_f_w2"][l], inputs["ffn_f_b2"][l]),
                (inputs["bwd_W"][l], inputs["bwd_b"][l], inputs["ln_b_g"][l],
                 inputs["ln_b_b"][l], inputs["ffn_b_w1"][l], inputs["ffn_b_b1"][l],
                 inputs["ffn_b_w2"][l], inputs["ffn_b_b2"][l]),
            )
        ):
            lb = l * 2 + br
            wpT[l, br] = np.asarray(Wc, f32).reshape(4, HC, 128, HC, 128)
            ctxb_col[lb] = np.asarray(bc, f32).reshape(HC, 128).T
            w1f = np.asarray(g, f32)[:, None] * np.asarray(w1, f32)
            b1f = np.asarray(b1, f32) + np.asarray(beta, f32) @ np.asarray(w1, f32)
            w1pT[l, br] = w1f.reshape(HC, 128, HC, 128)
            b1p[lb] = b1f.reshape(HC, 128).T
            cs1 = w1f.sum(0).reshape(HC, 128)  # colsum
            for gg in range(4):
                cs1_s[32 * gg, lb] = cs1
            w2T[l, br] = np.asarray(w2_, f32).reshape(HC, 128, HC, 128)
            b2col[lb] = np.asarray(b2, f32).reshape(HC, 128).T
        fp = np.asarray(inputs["fwd_pad"][l], f32)
        bp = np.asarray(inputs["bwd_pad"][l], f32)
        padT[l] = np.concatenate([fp, bp], 0).T.reshape(HC, 128, 2 * W)

    ones4 = np.zeros((128, 128), f32)
    for gg in range(4):
        ones4[32 * gg] = 1.0
    ohcols = np.zeros((4, 128, 128), f32)
    for gg in range(4):
        ohcols[gg, :, 32 * gg] = 1.0
    bf = NPBF
    return dict(
        wpT=wpT.astype(bf), ctxb_col=ctxb_col, w1pT=w1pT.astype(bf), b1p=b1p,
        cs1_s=cs1_s.astype(bf), w2T=w2T.astype(bf), b2col=b2col,
        padT=padT.astype(bf), ones4=ones4.astype(bf), ohcols=ohcols.astype(bf))


def build_nc(B_local, S_, L):
    NG = S_ // 512
    SP = S_ + 2 * W
    LB = 2 * L

    nc = bacc.Bacc()
    dr = {}
    dr["x0fm"] = nc.dram_tensor("x0fm", [128, HC, B_local, S_], BF16,
                                kind="ExternalInput")
    dr["wpT"] = nc.dram_tensor("wpT", [L, 2, 4, HC, 128, HC, 128], BF16,
                               kind="ExternalInput")
    dr["ctxb_col"] = nc.dram_tensor("ctxb_col", [LB, 128, HC], F32,
                                    kind="ExternalInput")
    dr["w1pT"] = nc.dram_tensor("w1pT", [L, 2, HC, 128, HC, 128], BF16,
                                kind="ExternalInput")
    dr["b1p"] = nc.dram_tensor("b1p", [LB, 128, HC], F32, kind="ExternalInput")
    dr["cs1_s"] = nc.dram_tensor("cs1_s", [128, LB, HC, 128], BF16,
                                 kind="ExternalInput")
    dr["w2T"] = nc.dram_tensor("w2T", [L, 2, HC, 128, HC, 128], BF16,
                               kind="ExternalInput")
    dr["b2col"] = nc.dram_tensor("b2col", [LB, 128, HC], F32, kind="ExternalInput")
    dr["padT"] = nc.dram_tensor("padT", [L, HC, 128, 2 * W], BF16,
                                kind="ExternalInput")
    dr["ones4"] = nc.dram_tensor("ones4", [128, 128], BF16, kind="ExternalInput")
    dr["ohcols"] = nc.dram_tensor("ohcols", [4, 128, 128], BF16,
                                  kind="ExternalInput")
    dr["out"] = nc.dram_tensor("out", [L, B_local, S_, 2 * H], BF16,
                               kind="ExternalOutput")

    with tile.TileContext(nc) as tc:
        _body(nc, tc, B_local, S_, L, NG, SP, LB, dr)
    nc.compile()
    return nc


def _body(nc, tc, B_local, S_, L, NG, SP, LB, dr):
    ctx = contextlib.ExitStack()
    with ctx:
        consts = ctx.enter_context(tc.tile_pool(name="consts", bufs=1))
        xbufs = ctx.enter_context(tc.tile_pool(name="xbufs", bufs=1))
        h_p = ctx.enter_context(tc.tile_pool(name="h", bufs=2))
        sq_p = ctx.enter_context(tc.tile_pool(name="sq", bufs=2))
        rows_p = ctx.enter_context(tc.tile_pool(name="rows", bufs=2))
        f1_p = ctx.enter_context(tc.tile_pool(name="f1", bufs=2))
        tmp_p = ctx.enter_context(tc.tile_pool(name="tmp", bufs=2))
        xn_p = ctx.enter_context(tc.tile_pool(name="xn", bufs=2))
        tm_p = ctx.enter_context(tc.tile_pool(name="tm", bufs=2))
        pm = ctx.enter_context(tc.tile_pool(name="pm", bufs=5, space="PSUM"))
        ps_st = ctx.enter_context(tc.tile_pool(name="ps_st", bufs=1, space="PSUM"))
        ps_tr = ctx.enter_context(tc.tile_pool(name="ps_tr", bufs=1, space="PSUM"))

        # ---- constants ----
        ident = consts.tile([128, 128], F32)
        make_identity(nc, ident[:])
        ident_b = consts.tile([128, 128], BF16)
        nc.vector.tensor_copy(out=ident_b[:], in_=ident[:])
        eps_t = consts.tile([128, 1], F32)
        nc.vector.memset(eps_t[:], EPS)
        ones4 = consts.tile([128, 128], BF16)
        nc.gpsimd.dma_start(ones4[:], dr["ones4"].ap())
        ohcols = consts.tile([128, 4, 128], BF16)
        nc.gpsimd.dma_start(ohcols[:], dr["ohcols"].ap().rearrange("g p m -> p g m"))
        cs1_s = consts.tile([128, LB, HC, 128], BF16)
        nc.gpsimd.dma_start(cs1_s[:], dr["cs1_s"].ap())
        ctxb_col = consts.tile([128, LB, HC], F32)
        nc.sync.dma_start(ctxb_col[:], dr["ctxb_col"].ap().rearrange("a p m -> p a m"))
        b1p = consts.tile([128, LB, HC], F32)
        nc.sync.dma_start(b1p[:], dr["b1p"].ap().rearrange("a p m -> p a m"))
        b2col = consts.tile([128, LB, HC], F32)
        nc.sync.dma_start(b2col[:], dr["b2col"].ap().rearrange("a p m -> p a m"))
        padT_s = consts.tile([128, L, HC, 2 * W], BF16)
        nc.gpsimd.dma_start(padT_s[:], dr["padT"].ap().rearrange("l c p w -> p l c w"))
        w1pT_s = consts.tile([128, L, 2, HC, HC, 128], BF16)
        nc.gpsimd.dma_start(
            w1pT_s[:], dr["w1pT"].ap().rearrange("l b c p m n -> p l b c m n"))
        w2T_s = consts.tile([128, L, 2, HC, HC, 128], BF16)
        nc.gpsimd.dma_start(
            w2T_s[:], dr["w2T"].ap().rearrange("l b c p m n -> p l b c m n"))
        # all proj weights resident (24KB/partition), loaded per-l at first use
        wpT_s = consts.tile([128, L, 2, 4, HC, HC, 128], BF16)

        # ---- per-batch buffer state ----
        bufs = {}   # b -> dict(x0, xA, xB)

        def in_buf(b, l, br):
            d = bufs[b]
            return d["x0"] if l == 0 else (d["xA"][br] if l % 2 == 1 else d["xB"][br])

        def out_buf(b, l, br):
            d = bufs[b]
            return d["xA"][br] if l % 2 == 0 else d["xB"][br]

        state = {}  # unit -> dict(h_sb, st_sum, st_sq, negm, rstd, sq_g, f1_g)

        def prologue(u):
            b, l, br = u
            if l == 0 and br == 0:
                d = {}
                d["x0"] = xbufs.tile([128, HC, SP], BF16, tag="xB0",
                                     name=f"x0_fm_{b}")
                d["xA"] = [xbufs.tile([128, HC, SP], BF16, tag=f"xA{i}",
                                      name=f"xA{i}_{b}") for i in range(2)]
                d["xB"] = [xbufs.tile([128, HC, SP], BF16, tag=f"xB{i}",
                                      name=f"xB{i}_{b}") for i in range(2)]
                bufs[b] = d
                nc.sync.dma_start(d["x0"][:, :, W : W + S_],
                                  dr["x0fm"].ap()[:, :, b, :])
            if b == 0 and br == 0:
                nc.gpsimd.dma_start(
                    wpT_s[:, l],
                    dr["wpT"].ap()[l].rearrange("b j c p m n -> p b j c m n"))
            if not (l == 0 and br == 1):
                buf = in_buf(b, l, br)
                nc.gpsimd.tensor_copy(buf[:, :, 0:W], padT_s[:, l, :, 0:W])
                nc.gpsimd.tensor_copy(
                    buf[:, :, S_ + W : S_ + 2 * W], padT_s[:, l, :, W : 2 * W])

        def emit_A(u, g):
            # context proj matmuls + relu; stats deferred to emit_stats
            b, l, br = u
            lb = l * 2 + br
            xin = in_buf(b, l, br)
            off = 0 if br == 0 else W
            st = state.setdefault(u, {})
            if g == 0:
                st["h_sb"] = h_p.tile([128, HC, S_], BF16, tag="h",
                                      name=f"h_{b}_{l}_{br}")
                st["st_sum"] = ps_st.tile([128, 512], F32, tag="st_sum",
                                          name=f"stsum_{b}_{l}_{br}")
                st["st_sq"] = ps_st.tile([128, 512], F32, tag="st_sq",
                                         name=f"stsq_{b}_{l}_{br}")
            h_sb = st["h_sb"]
            t0 = g * 512
            for m in range(HC):
                psc = pm.tile([128, 512], F32, tag="pm", name="psc")
                for j in range(W + 1):
                    for c in range(HC):
                        nc.tensor.matmul(
                            psc[:], wpT_s[:, l, br, j, c, m, :],
                            xin[:, c, t0 + off + j : t0 + off + j + 512],
                            start=(j == 0 and c == 0),
                            stop=(j == W and c == HC - 1))
                nc.scalar.activation(
                    h_sb[:, m, t0 : t0 + 512], psc[:], AF.Relu,
                    bias=ctxb_col[:, lb, m : m + 1])
            # square on DVE (bf16 2x), one op for both feature blocks
            sq = sq_p.tile([128, HC, 512], BF16, tag="sq", name="sq")
            nc.vector.tensor_tensor(
                out=sq[:], in0=h_sb[:, :, t0 : t0 + 512],
                in1=h_sb[:, :, t0 : t0 + 512], op=ALU.mult)
            st[("sq", g)] = sq

        def emit_stats(u, g):
            st = state[u]
            h_sb = st["h_sb"]
            sq = st.pop(("sq", g))
            t0 = g * 512
            for m in range(HC):
                nc.tensor.matmul(
                    st["st_sum"][:], ohcols[:, g % 4, :], h_sb[:, m, t0 : t0 + 512],
                    start=(g == 0 and m == 0), stop=(g == NG - 1 and m == HC - 1),
                    skip_group_check=True)
                nc.tensor.matmul(
                    st["st_sq"][:], ohcols[:, g % 4, :], sq[:, m, :],
                    start=(g == 0 and m == 0), stop=(g == NG - 1 and m == HC - 1),
                    skip_group_check=True)

        def emit_R(u):
            st = state[u]
            negm = rows_p.tile([128, 512], BF16, tag="negm", name="negm")
            nc.vector.tensor_scalar_mul(negm[:], st["st_sum"][:], -1.0 / H)
            m2 = rows_p.tile([128, 512], F32, tag="rtmp", name="m2")
            nc.vector.tensor_mul(m2[:], negm[:], negm[:])
            v = rows_p.tile([128, 512], F32, tag="rtmp2", name="v")
            nc.vector.scalar_tensor_tensor(
                out=v[:], in0=st["st_sq"][:], scalar=1.0 / H, in1=m2[:],
                op0=ALU.mult, op1=ALU.subtract)
            rstd = rows_p.tile([128, 512], BF16, tag="rstd", name="rstd")
            nc.scalar.activation(
                rstd[:], v[:], AF.Abs_reciprocal_sqrt, bias=eps_t[:])
            st["negm"] = negm
            st["rstd"] = rstd

        def emit_B1(u, g):
            b, l, br = u
            lb = l * 2 + br
            st = state[u]
            h_sb, negm, rstd = st["h_sb"], st["negm"], st["rstd"]
            t0 = g * 512
            gp = 32 * (g % 4)
            psz = [None, None]
            for m in range(HC):
                psz[m] = pm.tile([128, 512], F32, tag="pm", name="psz")
                for c in range(HC):
                    nc.tensor.matmul(
                        psz[m][:], w1pT_s[:, l, br, c, m, :],
                        h_sb[:, c, t0 : t0 + 512], start=(c == 0), stop=False)
                nc.tensor.matmul(
                    psz[m][:], cs1_s[gp : gp + 1, lb, m, :], negm[gp : gp + 1, :],
                    start=False, stop=True, tile_position=(gp, 0))
            rb = pm.tile([128, 512], F32, tag="pm", name="rb")
            nc.tensor.matmul(
                rb[:], ones4[gp : gp + 1, :], rstd[gp : gp + 1, :],
                start=True, stop=True, tile_position=(gp, 0))
            rb_sb = tmp_p.tile([128, 512], F32, tag="rb_sb", name="rb_sb")
            nc.scalar.copy(out=rb_sb[:], in_=rb[:])
            f1_sb = f1_p.tile([128, HC, 512], BF16, tag="f1", name="f1_sb")
            for m in range(HC):
                t_sb = tmp_p.tile([128, 512], BF16, tag="t_sb", name="t_sb")
                nc.vector.tensor_mul(t_sb[:], psz[m][:], rb_sb[:])
                nc.scalar.activation(
                    f1_sb[:, m, :], t_sb[:], AF.Relu, bias=b1p[:, lb, m : m + 1])
            st[("f1", g)] = f1_sb

        def emit_B2(u, g):
            b, l, br = u
            lb = l * 2 + br
            st = state[u]
            h_sb = st["h_sb"]
            f1_sb = st.pop(("f1", g))
            t0 = g * 512
            if l < L - 1:
                xb = out_buf(b, l, br)

                def xsl(m, a, n):
                    return xb[:, m, W + t0 + a : W + t0 + a + n]
            else:
                xn_t = xn_p.tile([128, HC, 512], BF16, tag="xn", name="xn_last")

                def xsl(m, a, n):
                    return xn_t[:, m, a : a + n]
            for m in range(HC):
                pso = pm.tile([128, 512], F32, tag="pm", name="pso")
                for c in range(HC):
                    nc.tensor.matmul(
                        pso[:], w2T_s[:, l, br, c, m, :], f1_sb[:, c, :],
                        start=(c == 0), stop=(c == HC - 1))
                nc.vector.scalar_tensor_tensor(
                    out=xsl(m, 0, 512), in0=pso[:],
                    scalar=b2col[:, lb, m : m + 1],
                    in1=h_sb[:, m, t0 : t0 + 512], op0=ALU.add, op1=ALU.add)
            # transpose to token-major: 8 x [128,128] bf16 into one psum bank
            tr = ps_tr.tile([128, 1024], BF16, tag="tr", name="tr")
            for s in range(4):
                for m in range(HC):
                    k = 2 * s + m
                    nc.tensor.transpose(
                        tr[:, k * 128 : (k + 1) * 128],
                        xsl(m, s * 128, 128),
                        ident_b[:])
            tmt = tm_p.tile([128, 1024], BF16, tag="tmt", name="tmt")
            if g % 2 == 0:
                nc.scalar.copy(out=tmt[:], in_=tr[:])
            else:
                nc.vector.tensor_copy(out=tmt[:], in_=tr[:])
            nc.sync.dma_start(
                dr["out"].ap()[l, b, t0 : t0 + 512, br * H : (br + 1) * H]
                .rearrange("(s p) f -> p s f", p=128),
                tmt[:].rearrange("p (s m n) -> p s (m n)", s=4, m=2))

        # ---- software-pipelined unit stream ----
        # PE order per group-iteration g:
        #   proj(u,g) | z+negm+rb(u-1,g) | stats(u,g-1) | ffn2+transpose(u-1,g-1)
        # stats lag one group so their ACT/DVE producers never stall PE; the
        # R row-chain of u hides behind proj(u+1, 0).
        units = [(b, l, br) for b in range(B_local) for l in range(L)
                 for br in range(2)]
        prev = None
        for u in units:
            prologue(u)
            for g in range(NG):
                emit_A(u, g)
                if prev is not None:
                    emit_B1(prev, g)
                if g > 0:
                    emit_stats(u, g - 1)
                    if prev is not None:
                        emit_B2(prev, g - 1)
            emit_stats(u, NG - 1)
            emit_R(u)
            if prev is not None:
                emit_B2(prev, NG - 1)
                state.pop(prev)
            prev = u
        for g in range(NG):
            emit_B1(prev, g)
            if g > 0:
                emit_B2(prev, g - 1)
        emit_B2(prev, NG - 1)


def ref_numpy(x0, inputs, L):
    B, S_, _ = x0.shape
    x_f = x_b = np.asarray(x0, np.float64)

    def branch(xpad, Wp, bp, g, beta, w1, b1, w2, b2, offs):
        ctxm = np.concatenate([xpad[:, k : k + S_] for k in offs], -1)
        h = np.maximum(ctxm @ Wp + bp, 0)
        m = h.mean(-1, keepdims=True)
        v = h.var(-1, keepdims=True)
        y = g * (h - m) / np.sqrt(v + EPS) + beta
        ffn = np.maximum(y @ w1 + b1, 0) @ w2 + b2
        return h + ffn

    outs = []
    I = {k: np.asarray(v, np.float64) for k, v in inputs.items() if k != "mask"}
    for l in range(L):
        fp = np.broadcast_to(I["fwd_pad"][l], (B, W, H))
        bp = np.broadcast_to(I["bwd_pad"][l], (B, W, H))
        pad_f = np.concatenate([fp, x_f, bp], 1)
        pad_b = np.concatenate([fp, x_b, bp], 1)
        x_f = branch(pad_f, I["fwd_W"][l], I["fwd_b"][l], I["ln_f_g"][l],
                     I["ln_f_b"][l], I["ffn_f_w1"][l], I["ffn_f_b1"][l],
                     I["ffn_f_w2"][l], I["ffn_f_b2"][l], range(W + 1))
        x_b = branch(pad_b, I["bwd_W"][l], I["bwd_b"][l], I["ln_b_g"][l],
                     I["ln_b_b"][l], I["ffn_b_w1"][l], I["ffn_b_b1"][l],
                     I["ffn_b_w2"][l], I["ffn_b_b2"][l], range(W, 2 * W + 1))
        outs.append(np.concatenate([x_f, x_b], -1))
    return np.stack(outs, 0)


# ---- SPMD wrapper ----
from concourse.bass_utils import run_bass_kernel_spmd

B, S, L_ = 32, 2048, 3
N_CORES = 8
B_LOCAL = B // N_CORES
_NC_CACHE = {}


def _get_nc():
    key = (B_LOCAL, S)
    if key not in _NC_CACHE:
        _NC_CACHE[key] = build_nc(B_LOCAL, S, L_)
    return _NC_CACHE[key]


def run(inputs, **spmd_kwargs):
    prep = prep_weights(inputs, L_)
    x = np.asarray(inputs["inputs"], np.float32)  # [B, S, H]
    # feature-major bf16: x0fm[p, c, b_local, t] per core
    xt = np.ascontiguousarray(
        x.reshape(B, S, HC, 128).transpose(3, 2, 0, 1)).astype(NPBF)
    nc = _get_nc()
    in_maps = []
    for core in range(N_CORES):
        m = {"x0fm": np.ascontiguousarray(
            xt[:, :, core * B_LOCAL : (core + 1) * B_LOCAL, :])}
        m.update(prep)
        in_maps.append(m)
    res = run_bass_kernel_spmd(nc, in_maps, list(range(N_CORES)), **spmd_kwargs)
    out = np.concatenate(
        [np.asarray(res.results[i]["out"]).astype(np.float32)
         for i in range(N_CORES)], axis=1)
    return out, res


def kernel(**inputs):
    out, _ = run(inputs)
    return out
